# revision 7
# baseline (speedup 1.0000x reference)
"""AttentiveChildSumTreeLSTMCell on 8 Trainium2 NeuronCores — restructured.

Tensor-parallel, hidden dim sharded 8 ways.  Three AllGathers (no AllReduce):
  AG1: partial attention logits + per-child f LayerNorm stat partials [96 f32]
  AG2: column-parallel merge-linear chunks [256 f32]
  AG3: iou chunk + f*cells chunk + iou LN stat partials [1032 f32]

Key differences from the previous version:
  - no warmup collective (the first real collective absorbs the comm-init
    barrier, which gates all collectives anyway)
  - W_merge is column-parallel with per-child speculative projections M
    computed before the logits arrive; ml chunk = exps-weighted reduce of M
    (softmax denominator cancels inside the merge LayerNorm)
  - "natural" [128,16] tile layout everywhere (sb[q,j] = v[q*16+j]) so
    AllGather outputs DMA straight into compute layout with 64B runs —
    no selector matmuls
  - single activation-table set: sigmoid via tanh half-angle (host halves
    the biases), rsqrt via DVE Newton with a bit-trick seed — the scalar
    engine only ever loads exp_and_others (exp/tanh) once
"""

import sys

for _p in ("/opt/trn_rl_repo",):
    if _p not in sys.path:
        sys.path.insert(0, _p)

import ml_dtypes
import numpy as np

import concourse.bacc as bacc
import concourse.mybir as mybir
import concourse.tile as tile
from concourse.bass_utils import run_bass_kernel_spmd
from concourse.tile_rust import add_dep_helper

F32 = mybir.dt.float32
BF16 = mybir.dt.bfloat16
I32 = mybir.dt.int32
FP8 = mybir.dt.float8e4
AF = mybir.ActivationFunctionType
ALU = mybir.AluOpType
NPBF = ml_dtypes.bfloat16
NPF8 = ml_dtypes.float8_e4m3fn

H = 2048
N = 32
NC = 8
S = H // NC           # 256 per-core chunk of every sharded dim
T = H // 128          # 16 tiles along a 2048 contraction/output dim
KT = 32               # k-tiles along the 4096 contraction dims
EPS = 1e-5
INV_H = 1.0 / H
MAGIC = 0x5F3759DF

_CACHE = {}


class _Chopped(Exception):
    pass


def _build(dbg=False):
    import os
    CHOP = int(os.environ.get("KB_CHOP", "0"))
    nc = bacc.Bacc(None, target_bir_lowering=False, debug=False, num_devices=NC)

    def din(name, shape, dt=F32):
        return nc.dram_tensor(name, list(shape), dt, kind="ExternalInput")

    # ---- per-core DRAM inputs (SPMD: same shapes on every core) ----
    hN = din("hN", (128, T * N), BF16)        # h[n, q*16+j] at [q, j*N+n]
    xN32 = din("xN32", (128, T * N), BF16)
    eN32 = din("eN32", (128, T * N), BF16)
    x1N = din("x1N", (128, T), BF16)
    cells_half = din("cells_half", (N, S))    # 0.5 * cells chunk
    gf_rep = din("gf_rep", (N, S))
    bf_half = din("bf_half", (N, S))
    wattn_rep = din("wattn_rep", (N, S))
    gm = din("gm", (128, T))
    bm = din("bm", (128, T))
    gi = din("gi", (128, T))
    bi_h = din("bi_h", (128, T))
    go = din("go", (128, T))
    bo_h = din("bo_h", (128, T))
    gu = din("gu", (128, T))
    bu = din("bu", (128, T))
    gc = din("gc", (128, T))
    bc = din("bc", (128, T))
    ones8 = din("ones8", (8, 1))
    ones32 = din("ones32", (N, 1))
    ones128 = din("ones128", (128, 1))
    onesr = din("onesr", (1, 128))
    ones11 = din("ones11", (1, 1))
    wai = din("wai", (128, KT * S), BF16)      # W_ai^T chunk, h|e k-tiles
    wf = din("wf", (128, KT * S), BF16)        # [W_fh | W_fi]^T chunk
    wmg = din("wmg", (128, T * S), BF16)       # W_merge^T col-chunk
    wiou = din("wiou", (128, KT * 3 * S), BF16)  # W_iou^T chunk, x|mh k-tiles

    out_h = nc.dram_tensor("out_h", [128, T], F32, kind="ExternalOutput")
    out_c = nc.dram_tensor("out_c", [128, T], F32, kind="ExternalOutput")
    dbg_t = {}
    if dbg:
        for nm, shp in [("d_s96", [1, 96]), ("d_ml", [128, T]),
                        ("d_mh", [128, T]), ("d_vec4", [128, 4 * T]),
                        ("d_f", [N, S]),
                        ("d_cl", [128, T]), ("d_rst", [1, 6])]:
            dbg_t[nm] = nc.dram_tensor(nm, shp, F32, kind="ExternalOutput")

    with tile.TileContext(nc) as tc:
        with (
            tc.tile_pool(name="sb", bufs=1) as sb,
            tc.tile_pool(name="ps", bufs=1, space="PSUM") as ps,
            tc.tile_pool(name="dram", bufs=1, space="DRAM") as dram,
        ):
            # ------- warmup collective: absorbs comm-init cold cost ----
            warm_in = dram.tile([1, 64], F32, name="warm_in")
            warm_out = dram.tile([8, 64], F32, name="warm_out")
            warm_sb = sb.tile([1, 64], F32, name="warm_sb")
            nc.vector.memset(warm_sb[:], 0.0)
            nc.sync.dma_start(warm_in[:], warm_sb[:])
            nc.gpsimd.collective_compute(
                "AllGather", ALU.bypass,
                replica_groups=[list(range(NC))],
                ins=[warm_in.opt()], outs=[warm_out.opt()])

            # ---------------- small resident loads ----------------
            def load(t_dram, shape, dt=F32):
                t_sb = sb.tile(shape, dt, name=t_dram.name + "_sb")
                nc.sync.dma_start(t_sb[:], t_dram[:])
                return t_sb

            hN_sb = load(hN, [128, T, N], BF16)
            xN32_sb = load(xN32, [128, T, N], BF16)
            eN32_sb = load(eN32, [128, T, N], BF16)
            x1N_sb = load(x1N, [128, T], BF16)
            cellsh_sb = load(cells_half, [N, S])
            gf_sb = load(gf_rep, [N, S])
            bfh_sb = load(bf_half, [N, S])
            wat_sb = load(wattn_rep, [N, S])
            gm_sb = load(gm, [128, T])
            bm_sb = load(bm, [128, T])
            gi_sb = load(gi, [128, T])
            bih_sb = load(bi_h, [128, T])
            go_sb = load(go, [128, T])
            boh_sb = load(bo_h, [128, T])
            gu_sb = load(gu, [128, T])
            bu_sb = load(bu, [128, T])
            gc_sb = load(gc, [128, T])
            bc_sb = load(bc, [128, T])
            ones8_sb = load(ones8, [8, 1])
            ones32_sb = load(ones32, [N, 1])
            ones128_sb = load(ones128, [128, 1])
            onesr_sb = load(onesr, [1, 128])
            ones11_sb = load(ones11, [1, 1])

            # preload the single activation table set (exp/tanh/square)
            tl_scr = sb.tile([1, 1], F32, name="tl_scr")
            nc.vector.memset(tl_scr[:], 0.5)
            nc.scalar.activation(tl_scr[:], tl_scr[:], AF.Exp)

            # table-free rsqrt: bit-trick seed (<=3.5% err) + Newton steps
            # on the DVE; 1 step -> <=1.8e-3 rel err, plenty for the 2e-2
            # budget and ~2us cheaper than an ACT_TABLE_LOAD round-trip
            def rsqrt_dve(out, x_ap, shape, nm, iters=1):
                t = sb.tile(shape, F32, name=nm + "_t")
                nc.vector.tensor_scalar(
                    out[:].bitcast(I32), x_ap.bitcast(I32), 1, -1,
                    op0=ALU.logical_shift_right, op1=ALU.bitwise_xor)
                nc.vector.tensor_scalar_add(out[:].bitcast(I32),
                                            out[:].bitcast(I32), MAGIC + 1)
                for _ in range(iters):
                    nc.vector.tensor_tensor(t[:], out[:], out[:], op=ALU.mult)
                    nc.vector.tensor_tensor(t[:], t[:], x_ap, op=ALU.mult)
                    nc.vector.tensor_scalar(t[:], t[:], -0.5, 1.5,
                                            op0=ALU.mult, op1=ALU.add)
                    nc.vector.tensor_tensor(out[:], out[:], t[:], op=ALU.mult)

            # ---------------- weight streaming DMAs (ordered) ----------------
            wai_sb = sb.tile([128, KT * S], BF16, name="wai_sb")
            wf_sb = sb.tile([128, KT * S], BF16, name="wf_sb")
            wmg_sb = sb.tile([128, T * S], BF16, name="wmg_sb")
            wiou_sb = sb.tile([128, KT * 3 * S], BF16, name="wiou_sb")

            wdmas = []
            for k in range(2):  # wai: 2 x 1MB
                wdmas.append(nc.sync.dma_start(
                    wai_sb[:, k * 4096:(k + 1) * 4096],
                    wai[:, k * 4096:(k + 1) * 4096]))
            for k in range(2):  # wf: 2 x 1MB
                wdmas.append(nc.sync.dma_start(
                    wf_sb[:, k * 4096:(k + 1) * 4096],
                    wf[:, k * 4096:(k + 1) * 4096]))
            wdmas.append(nc.sync.dma_start(wmg_sb[:], wmg[:]))  # 1MB
            for k in range(6):  # wiou: x half then mh half, 6 x 1MB
                wdmas.append(nc.sync.dma_start(
                    wiou_sb[:, k * 4096:(k + 1) * 4096],
                    wiou[:, k * 4096:(k + 1) * 4096]))
            for i in range(2, len(wdmas)):
                add_dep_helper(wdmas[i].ins, wdmas[i - 2].ins, sync=True,
                               reason="weight DMA arrival order")

            # ---------------- attention: ai, partial logits ----------------
            ps_ai = ps.tile([N, S], F32, name="ps_ai", tag="pA")
            for kt in range(KT):
                act = hN_sb if kt < T else eN32_sb
                nc.tensor.matmul(ps_ai[:], act[:, kt % T, :],
                                 wai_sb[:, kt * S:(kt + 1) * S],
                                 start=(kt == 0), stop=(kt == KT - 1))
            ai_sb = sb.tile([N, S], F32, name="ai_sb")
            nc.scalar.activation(ai_sb[:], ps_ai[:], AF.Tanh)
            aw_sb = sb.tile([N, S], F32, name="aw_sb")
            st3 = sb.tile([N, 3], F32, name="st3")
            nc.vector.tensor_tensor(aw_sb[:], ai_sb[:], wat_sb[:], op=ALU.mult)
            nc.vector.tensor_reduce(st3[:, 0:1], aw_sb[:],
                                    mybir.AxisListType.X, ALU.add)

            # ---------------- f_lin + per-child stat partials ----------------
            ps_f = ps.tile([N, S], F32, name="ps_f", tag="pB")
            for kt in range(KT):
                act = hN_sb if kt < T else xN32_sb
                nc.tensor.matmul(ps_f[:], act[:, kt % T, :],
                                 wf_sb[:, kt * S:(kt + 1) * S],
                                 start=(kt == 0), stop=(kt == KT - 1))
            f_lin_sb = sb.tile([N, S], F32, name="f_lin_sb")
            fsq_scr = sb.tile([N, S], F32, name="fsq_scr")
            nc.vector.tensor_copy(f_lin_sb[:], ps_f[:])
            nc.vector.tensor_reduce(st3[:, 1:2], f_lin_sb[:],
                                    mybir.AxisListType.X, ALU.add)
            nc.vector.scalar_tensor_tensor(fsq_scr[:], f_lin_sb[:], 1.0,
                                           f_lin_sb[:], op0=ALU.mult,
                                           op1=ALU.mult,
                                           accum_out=st3[:, 2:3])

            # speculative per-child merge projections (col-parallel W_merge):
            # M[n, s] = sum_k h[n, k] * W_merge[c*S+s, k]
            ps_M = ps.tile([N, S], F32, name="ps_M", tag="pC")
            for kt in range(T):
                nc.tensor.matmul(ps_M[:], hN_sb[:, kt, :],
                                 wmg_sb[:, kt * S:(kt + 1) * S],
                                 start=(kt == 0), stop=(kt == T - 1))
            M_sb = sb.tile([N, S], F32, name="M_sb")
            nc.vector.tensor_copy(M_sb[:], ps_M[:])

            # iou x-half: runs as soon as those weights land (PE idle time)
            ps_iou = ps.tile([1, 3 * S], F32, name="ps_iou", tag="pIOU")
            nslices = ((0, 512), (512, 768))

            def iou_mm(kt, lhs, start, stop):
                for c0, c1 in nslices:
                    nc.tensor.matmul(ps_iou[:, c0:c1], lhs,
                                     wiou_sb[:, kt * 768 + c0:kt * 768 + c1],
                                     start=start, stop=stop)

            for kt in range(T):
                iou_mm(kt, x1N_sb[:, kt:kt + 1], kt == 0, False)

            # ---------------- AG1: logits + f stats (96 floats) ----------------
            ag1_in = dram.tile([1, 3 * N], F32, name="ag1_in")
            ag1_out = dram.tile([8, 3 * N], F32, name="ag1_out")
            nc.gpsimd.dma_start(
                ag1_in[0, :].rearrange("(k n) -> n k", n=N), st3[:])
            nc.gpsimd.collective_compute(
                "AllGather", ALU.bypass,
                replica_groups=[list(range(NC))],
                ins=[ag1_in.opt()], outs=[ag1_out.opt()])
            ag1_sb = sb.tile([8, 3 * N], F32, name="ag1_sb")
            nc.sync.dma_start(ag1_sb[:], ag1_out[:])

            if CHOP == 1:
                zz = sb.tile([128, T], F32, name="zz")
                nc.vector.memset(zz[:], 0.0)
                nc.vector.tensor_copy(zz[0:8, 0:12], ag1_sb[:, 0:12])
                nc.sync.dma_start(out_c[:], zz[:])
                nc.sync.dma_start(out_h[:], zz[:])
                raise _Chopped()

            # sum partials across cores -> [1, 96] = [logits | fsum | fss]
            ps96 = ps.tile([1, 3 * N], F32, name="ps96", tag="pA")
            nc.tensor.matmul(ps96[:], ones8_sb[:], ag1_sb[:],
                             start=True, stop=True)
            # softmax without max-subtraction or normalization: the scale
            # cancels inside the merge LayerNorm
            exps_row = sb.tile([1, N], F32, name="exps_row")
            nc.scalar.activation(exps_row[:], ps96[:, 0:N], AF.Exp)
            s96 = sb.tile([1, 3 * N], F32, name="s96")
            nc.vector.tensor_copy(s96[:], ps96[:])
            # transpose [1,32] -> [32,1] via a K=1 matmul
            ps_e32 = ps.tile([N, 1], F32, name="ps_e32", tag="pE")
            nc.tensor.matmul(ps_e32[:], exps_row[:], ones11_sb[:],
                             start=True, stop=True)
            e32 = sb.tile([N, 1], F32, name="e32")
            nc.vector.tensor_copy(e32[:], ps_e32[:])

            # ml chunk = sum_n exps[n] * M[n, :] as one K=32 matmul
            ps_ml = ps.tile([1, S], F32, name="ps_ml", tag="pC")
            nc.tensor.matmul(ps_ml[:], e32[:], M_sb[:],
                             start=True, stop=True)
            pay = sb.tile([1, 272], F32, name="pay")
            nc.vector.memset(pay[:], 0.0)
            pay3 = pay[:, :].rearrange("one (ql j17) -> one ql j17", j17=17)
            nc.vector.tensor_copy(
                pay3[:, :, 0:16],
                ps_ml[:, :].rearrange("one (ql j) -> one ql j", ql=16))
            mlsq = sb.tile([1, S], F32, name="mlsq")
            nc.vector.tensor_reduce(pay[:, 16:17],
                                    ps_ml[:], mybir.AxisListType.X, ALU.add)
            mlv = pay3[:, :, 0:16]
            nc.vector.scalar_tensor_tensor(
                mlsq[:, :].rearrange("one (a b) -> one a b", a=16),
                mlv, 1.0, mlv, op0=ALU.mult, op1=ALU.mult,
                accum_out=pay[:, 33:34])

            # ---------------- AG2: merge-linear chunks + stats ----------------
            ag2_in = dram.tile([1, 272], F32, name="ag2_in")
            ag2_out = dram.tile([8, 272], F32, name="ag2_out")
            nc.gpsimd.dma_start(ag2_in[:], pay[:])
            nc.gpsimd.collective_compute(
                "AllGather", ALU.bypass,
                replica_groups=[list(range(NC))],
                ins=[ag2_in.opt()], outs=[ag2_out.opt()])
            ml_t = sb.tile([128, T], F32, name="ml_t")
            nc.sync.dma_start(
                ml_t[:],
                ag2_out[:].rearrange("qh w -> (qh w)").rearrange(
                    "(qh ql j) -> (qh ql) j", qh=8, ql=16, j=17)[:, 0:16])
            st82 = sb.tile([8, 2], F32, name="st82")
            nc.gpsimd.dma_start(st82[:], ag2_out[:, 16:34:17])

            if CHOP == 2:
                zz = sb.tile([128, T], F32, name="zz")
                nc.vector.tensor_copy(zz[:], ml_t[:])
                nc.sync.dma_start(out_c[:], zz[:])
                nc.sync.dma_start(out_h[:], zz[:])
                raise _Chopped()

            # ---- f gate (off critical path): tanh half-angle sigmoid ----
            fmean = sb.tile([N, 1], F32, name="fmean")
            fvar = sb.tile([N, 1], F32, name="fvar")
            frsth = sb.tile([N, 1], F32, name="frsth")
            ps_f3 = ps.tile([N, 2], F32, name="ps_f3", tag="pC")
            nc.tensor.matmul(ps_f3[:, 0:1], s96[:, N:2 * N], ones11_sb[:],
                             start=True, stop=True)
            nc.tensor.matmul(ps_f3[:, 1:2], s96[:, 2 * N:3 * N], ones11_sb[:],
                             start=True, stop=True)
            nc.vector.tensor_scalar_mul(fmean[:], ps_f3[:, 0:1], INV_H)
            nc.vector.tensor_scalar_mul(fvar[:], ps_f3[:, 1:2], INV_H)
            fm2 = sb.tile([N, 1], F32, name="fm2")
            nc.vector.tensor_tensor(fm2[:], fmean[:], fmean[:], op=ALU.mult)
            nc.vector.tensor_sub(fvar[:], fvar[:], fm2[:])
            nc.vector.tensor_scalar_add(fvar[:], fvar[:], EPS)
            rsqrt_dve(frsth, fvar[:], [N, 1], "frs")
            nc.vector.tensor_scalar_mul(frsth[:], frsth[:], 0.5)
            ft = sb.tile([N, S], F32, name="ft")
            nc.vector.tensor_scalar(ft[:], f_lin_sb[:], fmean[:], frsth[:],
                                    op0=ALU.subtract, op1=ALU.mult)
            nc.vector.tensor_tensor(ft[:], ft[:], gf_sb[:], op=ALU.mult)
            nc.vector.tensor_tensor(ft[:], ft[:], bfh_sb[:], op=ALU.add)
            tf_sb = sb.tile([N, S], F32, name="tf_sb")
            nc.scalar.activation(tf_sb[:], ft[:], AF.Tanh)
            # fprod = 0.5*(1+tanh)*cells = (tanh + 1) * cells_half
            fprod = sb.tile([N, S], F32, name="fprod")
            nc.vector.scalar_tensor_tensor(fprod[:], tf_sb[:], 1.0,
                                           cellsh_sb[:], op0=ALU.add,
                                           op1=ALU.mult)
            # AG3 payload, per-rank order (ql, g, j) so the gathered result
            # DMAs straight into [128, (g j)] compute layout
            big = sb.tile([1, 1024], F32, name="big")
            big4 = big[:, :].rearrange("one (ql g j) -> one ql g j",
                                       ql=16, g=4)
            ps_fc = ps.tile([1, S], F32, name="ps_fc", tag="pB")
            nc.tensor.matmul(ps_fc[:], ones32_sb[:], fprod[:],
                             start=True, stop=True)
            nc.vector.tensor_copy(
                big4[:, :, 3, :],
                ps_fc[:, :].rearrange("one (ql j) -> one ql j", ql=16))

            # ---------------- post-AG2: merge LN + mh + iou mh-half ----------
            # global stats arrive via the payload slots; runs concurrently
            # with the ml gather DMA
            ps_sm = ps.tile([1, 2], F32, name="ps_sm", tag="pC")
            nc.tensor.matmul(ps_sm[:], ones8_sb[:], st82[:],
                             start=True, stop=True)
            mrm = sb.tile([1, 2], F32, name="mrm")
            nc.vector.tensor_scalar_mul(mrm[:, 0:1], ps_sm[:, 0:1], INV_H)
            mvar = sb.tile([1, 1], F32, name="mvar")
            mm2 = sb.tile([1, 1], F32, name="mm2")
            nc.vector.tensor_scalar_mul(mvar[:], ps_sm[:, 1:2], INV_H)
            nc.vector.tensor_tensor(mm2[:], mrm[:, 0:1], mrm[:, 0:1],
                                    op=ALU.mult)
            nc.vector.tensor_sub(mvar[:], mvar[:], mm2[:])
            mrst = sb.tile([1, 1], F32, name="mrst")
            rsqrt_dve(mrst, mvar[:], [1, 1], "mrs")
            nc.vector.tensor_copy(mrm[:, 1:2], mrst[:])
            ps_bm = ps.tile([128, 2], F32, name="ps_bm", tag="pE")
            nc.tensor.matmul(ps_bm[:], onesr_sb[:], mrm[:],
                             start=True, stop=True)
            mrbm = sb.tile([128, 2], F32, name="mrbm")
            nc.vector.tensor_copy(mrbm[:], ps_bm[:])
            mt1 = sb.tile([128, T], F32, name="mt1")
            nc.vector.tensor_scalar(mt1[:], ml_t[:], mrbm[:, 0:1],
                                    mrbm[:, 1:2], op0=ALU.subtract,
                                    op1=ALU.mult)
            nc.vector.tensor_tensor(mt1[:], mt1[:], gm_sb[:], op=ALU.mult)
            nc.vector.tensor_tensor(mt1[:], mt1[:], bm_sb[:], op=ALU.add)
            mh_bf = sb.tile([128, T], BF16, name="mh_bf")
            nc.scalar.activation(mh_bf[:], mt1[:], AF.Tanh)

            # iou mh-half: the only post-AG2 PE work
            for kt in range(T, KT):
                iou_mm(kt, mh_bf[:, kt - T:kt - T + 1], False, kt == KT - 1)
            nc.vector.tensor_copy(
                big4[:, :, 0:3, :],
                ps_iou[:, :].rearrange("one (g ql j) -> one ql g j",
                                       g=3, ql=16))

            # ---------------- AG3: iou + fc (1024 floats) ------------
            ag3_in = dram.tile([1, 1024], F32, name="ag3_in")
            ag3_out = dram.tile([8, 1024], F32, name="ag3_out")
            nc.gpsimd.dma_start(ag3_in[:], big[:])
            nc.gpsimd.collective_compute(
                "AllGather", ALU.bypass,
                replica_groups=[list(range(NC))],
                ins=[ag3_in.opt()], outs=[ag3_out.opt()])

            # gather straight into compute layout: [128, 64] = i|o|u|fc
            vec4 = sb.tile([128, 4 * T], F32, name="vec4")
            nc.sync.dma_start(
                vec4[:],
                ag3_out[:].rearrange("qh w -> (qh w)").rearrange(
                    "(qh ql g j) -> (qh ql) (g j)", qh=8, ql=16, g=4, j=16))

            if CHOP == 3:
                zz = sb.tile([128, T], F32, name="zz")
                nc.vector.tensor_copy(zz[:], vec4[:, 0:T])
                nc.sync.dma_start(out_c[:], zz[:])
                nc.sync.dma_start(out_h[:], zz[:])
                raise _Chopped()

            def vg(g):
                return vec4[:, g * T:(g + 1) * T]

            # global iou stats from the gathered values
            st6 = sb.tile([128, 6], F32, name="st6")
            sq6 = sb.tile([128, T], F32, name="sq6")
            for g in range(3):
                nc.vector.tensor_reduce(st6[:, 2 * g:2 * g + 1], vg(g),
                                        mybir.AxisListType.X, ALU.add)
                nc.vector.scalar_tensor_tensor(
                    sq6[:], vg(g), 1.0, vg(g), op0=ALU.mult, op1=ALU.mult,
                    accum_out=st6[:, 2 * g + 1:2 * g + 2])
            ps6 = ps.tile([1, 6], F32, name="ps6", tag="pA")
            nc.tensor.matmul(ps6[:], ones128_sb[:], st6[:],
                             start=True, stop=True)
            m3 = sb.tile([1, 3], F32, name="m3")
            v3 = sb.tile([1, 3], F32, name="v3")
            m32 = sb.tile([1, 3], F32, name="m32")
            nc.vector.tensor_scalar_mul(m3[:], ps6[:, 0:6:2], INV_H)
            nc.vector.tensor_scalar_mul(v3[:], ps6[:, 1:6:2], INV_H)
            nc.vector.tensor_tensor(m32[:], m3[:], m3[:], op=ALU.mult)
            nc.vector.tensor_sub(v3[:], v3[:], m32[:])
            r3 = sb.tile([1, 3], F32, name="r3")
            rsqrt_dve(r3, v3[:], [1, 3], "r3")
            # halve rstd for i and o (tanh half-angle sigmoid)
            mr6 = sb.tile([1, 6], F32, name="mr6")
            nc.vector.tensor_copy(mr6[:, 0:6:2], m3[:])
            nc.vector.tensor_scalar_mul(mr6[:, 1:4:2], r3[:, 0:2], 0.5)
            nc.vector.tensor_copy(mr6[:, 5:6], r3[:, 2:3])
            ps_b6 = ps.tile([128, 6], F32, name="ps_b6", tag="pB")
            nc.tensor.matmul(ps_b6[:], onesr_sb[:], mr6[:],
                             start=True, stop=True)
            mrb6 = sb.tile([128, 6], F32, name="mrb6")
            nc.vector.tensor_copy(mrb6[:], ps_b6[:])

            def gate_apply(g, g_t, b_t, nm):
                t1 = sb.tile([128, T], F32, name=nm + "_t1")
                nc.vector.tensor_scalar(t1[:], vg(g),
                                        mrb6[:, 2 * g:2 * g + 1],
                                        mrb6[:, 2 * g + 1:2 * g + 2],
                                        op0=ALU.subtract, op1=ALU.mult)
                nc.vector.tensor_tensor(t1[:], t1[:], g_t[:], op=ALU.mult)
                nc.vector.tensor_tensor(t1[:], t1[:], b_t[:], op=ALU.add)
                out = sb.tile([128, T], F32, name=nm)
                nc.scalar.activation(out[:], t1[:], AF.Tanh)
                return out

            ti_sb = gate_apply(0, gi_sb, bih_sb, "tig")
            to_sb = gate_apply(1, go_sb, boh_sb, "tog")
            u_sb = gate_apply(2, gu_sb, bu_sb, "ug")

            # cell_lin = i*u + fc, i = 0.5*(1+ti)
            cl2 = sb.tile([128, T], F32, name="cl2")
            nc.vector.scalar_tensor_tensor(cl2[:], ti_sb[:], 1.0, u_sb[:],
                                           op0=ALU.add, op1=ALU.mult)
            cell_lin = sb.tile([128, T], F32, name="cell_lin")
            nc.vector.scalar_tensor_tensor(cell_lin[:], cl2[:], 0.5, vg(3),
                                           op0=ALU.mult, op1=ALU.add)

            # cell LayerNorm
            st2c = sb.tile([128, 2], F32, name="st2c")
            sqc = sb.tile([128, T], F32, name="sqc")
            nc.vector.tensor_reduce(st2c[:, 0:1], cell_lin[:],
                                    mybir.AxisListType.X, ALU.add)
            nc.vector.scalar_tensor_tensor(sqc[:], cell_lin[:], 1.0,
                                           cell_lin[:], op0=ALU.mult,
                                           op1=ALU.mult,
                                           accum_out=st2c[:, 1:2])
            ps_sc = ps.tile([1, 2], F32, name="ps_sc", tag="pC")
            nc.tensor.matmul(ps_sc[:], ones128_sb[:], st2c[:],
                             start=True, stop=True)
            mrc = sb.tile([1, 2], F32, name="mrc")
            nc.vector.tensor_scalar_mul(mrc[:, 0:1], ps_sc[:, 0:1], INV_H)
            cvar = sb.tile([1, 1], F32, name="cvar")
            cm2 = sb.tile([1, 1], F32, name="cm2")
            nc.vector.tensor_scalar_mul(cvar[:], ps_sc[:, 1:2], INV_H)
            nc.vector.tensor_tensor(cm2[:], mrc[:, 0:1], mrc[:, 0:1],
                                    op=ALU.mult)
            nc.vector.tensor_sub(cvar[:], cvar[:], cm2[:])
            crst = sb.tile([1, 1], F32, name="crst")
            rsqrt_dve(crst, cvar[:], [1, 1], "crs")
            nc.vector.tensor_copy(mrc[:, 1:2], crst[:])
            ps_bc2 = ps.tile([128, 2], F32, name="ps_bc2", tag="pE")
            nc.tensor.matmul(ps_bc2[:], onesr_sb[:], mrc[:],
                             start=True, stop=True)
            mrbc = sb.tile([128, 2], F32, name="mrbc")
            nc.vector.tensor_copy(mrbc[:], ps_bc2[:])
            ct1 = sb.tile([128, T], F32, name="ct1")
            nc.vector.tensor_scalar(ct1[:], cell_lin[:], mrbc[:, 0:1],
                                    mrbc[:, 1:2], op0=ALU.subtract,
                                    op1=ALU.mult)
            nc.vector.tensor_tensor(ct1[:], ct1[:], gc_sb[:], op=ALU.mult)
            new_c = sb.tile([128, T], F32, name="new_c")
            nc.vector.tensor_tensor(new_c[:], ct1[:], bc_sb[:], op=ALU.add)
            nc.sync.dma_start(out_c[:], new_c[:])

            th = sb.tile([128, T], F32, name="th")
            nc.scalar.activation(th[:], new_c[:], AF.Tanh)
            # new_h = o * th, o = 0.5*(1+to)
            nh2 = sb.tile([128, T], F32, name="nh2")
            nc.vector.scalar_tensor_tensor(nh2[:], to_sb[:], 1.0, th[:],
                                           op0=ALU.add, op1=ALU.mult)
            nc.sync.dma_start(out_h[:], nh2[:])  # = 2*new_h; halved on host

            if dbg:
                def dump(nm, src, shape):
                    dd = sb.tile(shape, F32, name=nm + "_d")
                    nc.vector.tensor_copy(dd[:], src)
                    nc.sync.dma_start(dbg_t[nm][:], dd[:])
                dump("d_s96", s96[:], [1, 96])
                dump("d_ml", ml_t[:], [128, T])
                dump("d_mh", mh_bf[:], [128, T])
                dump("d_vec4", vec4[:], [128, 4 * T])
                dump("d_f", tf_sb[:], [N, S])
                dump("d_cl", cell_lin[:], [128, T])
                dump("d_rst", mr6[:], [1, 6])

    nc.compile()
    return nc


def _nat(v):
    """[2048] vector -> natural [128,16] image (sb[q,j] = v[q*16+j])."""
    return np.ascontiguousarray(np.asarray(v, np.float32).reshape(128, T))


def _kpack(w_out_in, segs):
    """Pack W[out_cols, K] (torch layout) as rhs k-tiles [128, n_tiles*cols].

    segs: list of (k0, k1) contraction segments, each of length 2048; tile
    j within a segment covers contraction rows {k0 + q*16 + j : q}.
    """
    cols = w_out_in.shape[0]
    parts = []
    for (k0, k1) in segs:
        wt = np.ascontiguousarray(w_out_in[:, k0:k1].T)  # [2048, cols]
        parts.append(wt.reshape(128, T, cols))
    arr = np.concatenate(parts, axis=1)  # [128, n_tiles, cols]
    return np.ascontiguousarray(arr.reshape(128, -1)).astype(NPBF)


def kernel(input, hiddens, cells, external,
           W_ai, W_attn, W_merge, W_iou, W_fi, W_fh,
           g_merge, b_merge, g_f, b_f, g_i, b_i, g_o, b_o, g_u, b_u,
           g_c, b_c, _dbg=False):
    key = ("nc", _dbg)
    if key not in _CACHE:
        _CACHE[key] = _build(_dbg)
    nc = _CACHE[key]

    f32 = np.float32
    input = np.asarray(input, f32)
    hiddens = np.asarray(hiddens, f32)
    cells = np.asarray(cells, f32)
    external = np.asarray(external, f32)
    W_ai = np.asarray(W_ai, f32)
    W_attn = np.asarray(W_attn, f32)
    W_merge = np.asarray(W_merge, f32)
    W_iou = np.asarray(W_iou, f32)
    W_fi = np.asarray(W_fi, f32)
    W_fh = np.asarray(W_fh, f32)

    # natural k-tile packs of the replicated activations
    hNp = np.ascontiguousarray(
        hiddens.T.reshape(128, T, N)).astype(NPBF).reshape(128, T * N)
    xN32p = np.ascontiguousarray(
        np.tile(input[:, None], (1, N)).reshape(128, T, N)
    ).astype(NPBF).reshape(128, T * N)
    eN32p = np.ascontiguousarray(
        np.tile(external[:, None], (1, N)).reshape(128, T, N)
    ).astype(NPBF).reshape(128, T * N)

    com = {
        "hN": hNp, "xN32": xN32p, "eN32": eN32p,
        "x1N": input.reshape(128, T).astype(NPBF),
        "gm": _nat(g_merge), "bm": _nat(b_merge),
        "gi": _nat(g_i), "bi_h": _nat(np.asarray(b_i, f32) * 0.5),
        "go": _nat(g_o), "bo_h": _nat(np.asarray(b_o, f32) * 0.5),
        "gu": _nat(g_u), "bu": _nat(b_u),
        "gc": _nat(g_c), "bc": _nat(b_c),
        "ones8": np.ones((8, 1), f32), "ones32": np.ones((N, 1), f32),
        "ones128": np.ones((128, 1), f32), "onesr": np.ones((1, 128), f32),
        "ones11": np.ones((1, 1), f32),
    }

    Wf_cat = np.concatenate([W_fh, W_fi], axis=1)              # [H, 4096]
    in_maps = []
    for c in range(NC):
        r = slice(c * S, (c + 1) * S)
        iou_rows = np.concatenate(
            [W_iou[g * H + c * S:g * H + (c + 1) * S, :] for g in range(3)],
            axis=0)                                            # [768, 4096]
        m = dict(com)
        m.update({
            "cells_half": np.ascontiguousarray(0.5 * cells[:, r]),
            "gf_rep": np.tile(np.asarray(g_f, f32)[r], (N, 1)),
            "bf_half": np.tile(np.asarray(b_f, f32)[r] * 0.5, (N, 1)),
            "wattn_rep": np.tile(W_attn[0, r], (N, 1)),
            "wai": _kpack(W_ai[r], [(0, H), (H, 2 * H)]),
            "wf": _kpack(Wf_cat[r], [(0, H), (H, 2 * H)]),
            "wmg": _kpack(W_merge[r], [(0, H)]),
            "wiou": _kpack(iou_rows, [(0, H), (H, 2 * H)]),
        })
        in_maps.append({k: (np.ascontiguousarray(v)
                            if v.dtype in (NPBF, NPF8)
                            else np.ascontiguousarray(v, f32))
                        for k, v in m.items()})

    res = run_bass_kernel_spmd(nc, in_maps, core_ids=list(range(NC)))
    _CACHE["last_results"] = res
    r0 = res.results[0]
    new_h = (r0["out_h"].reshape(H) * 0.5).astype(f32)
    new_c = r0["out_c"].reshape(H).astype(f32)
    if _dbg:
        _CACHE["dbg"] = {k: r0[k] for k in r0 if k.startswith("d_")}
    return new_h, new_c


# revision 8
# speedup vs baseline: 1.0577x; 1.0577x over previous
"""AttentiveChildSumTreeLSTMCell on 8 Trainium2 NeuronCores — restructured.

Tensor-parallel, hidden dim sharded 8 ways.  Three AllGathers (no AllReduce):
  AG1: partial attention logits + per-child f LayerNorm stat partials [96 f32]
  AG2: column-parallel merge-linear chunks [256 f32]
  AG3: iou chunk + f*cells chunk + iou LN stat partials [1032 f32]

Key differences from the previous version:
  - no warmup collective (the first real collective absorbs the comm-init
    barrier, which gates all collectives anyway)
  - W_merge is column-parallel with per-child speculative projections M
    computed before the logits arrive; ml chunk = exps-weighted reduce of M
    (softmax denominator cancels inside the merge LayerNorm)
  - "natural" [128,16] tile layout everywhere (sb[q,j] = v[q*16+j]) so
    AllGather outputs DMA straight into compute layout with 64B runs —
    no selector matmuls
  - single activation-table set: sigmoid via tanh half-angle (host halves
    the biases), rsqrt via DVE Newton with a bit-trick seed — the scalar
    engine only ever loads exp_and_others (exp/tanh) once
"""

import sys

for _p in ("/opt/trn_rl_repo",):
    if _p not in sys.path:
        sys.path.insert(0, _p)

import ml_dtypes
import numpy as np

import concourse.bacc as bacc
import concourse.mybir as mybir
import concourse.tile as tile
from concourse.bass_utils import run_bass_kernel_spmd
from concourse.tile_rust import add_dep_helper

F32 = mybir.dt.float32
BF16 = mybir.dt.bfloat16
I32 = mybir.dt.int32
FP8 = mybir.dt.float8e4
AF = mybir.ActivationFunctionType
ALU = mybir.AluOpType
NPBF = ml_dtypes.bfloat16
NPF8 = ml_dtypes.float8_e4m3fn

H = 2048
N = 32
NC = 8
S = H // NC           # 256 per-core chunk of every sharded dim
T = H // 128          # 16 tiles along a 2048 contraction/output dim
KT = 32               # k-tiles along the 4096 contraction dims
EPS = 1e-5
INV_H = 1.0 / H
MAGIC = 0x5F3759DF

_CACHE = {}


class _Chopped(Exception):
    pass


def _build(dbg=False):
    import os
    CHOP = int(os.environ.get("KB_CHOP", "0"))
    nc = bacc.Bacc(None, target_bir_lowering=False, debug=False, num_devices=NC)

    def din(name, shape, dt=F32):
        return nc.dram_tensor(name, list(shape), dt, kind="ExternalInput")

    # ---- per-core DRAM inputs (SPMD: same shapes on every core) ----
    hN = din("hN", (128, T * N), BF16)        # h[n, q*16+j] at [q, j*N+n]
    xN32 = din("xN32", (128, T * N), BF16)
    eN32 = din("eN32", (128, T * N), BF16)
    x1N = din("x1N", (128, T), BF16)
    cells_half = din("cells_half", (N, S))    # 0.5 * cells chunk
    gf_rep = din("gf_rep", (N, S))
    bf_half = din("bf_half", (N, S))
    wattn_rep = din("wattn_rep", (N, S))
    gm = din("gm", (128, T))
    bm = din("bm", (128, T))
    gi = din("gi", (128, T))
    bi_h = din("bi_h", (128, T))
    go = din("go", (128, T))
    bo_h = din("bo_h", (128, T))
    gu = din("gu", (128, T))
    bu = din("bu", (128, T))
    gc = din("gc", (128, T))
    bc = din("bc", (128, T))
    ones8 = din("ones8", (8, 1))
    ones32 = din("ones32", (N, 1))
    ones128 = din("ones128", (128, 1))
    onesr = din("onesr", (1, 128))
    ones11 = din("ones11", (1, 1))
    wai = din("wai", (128, KT * S), BF16)      # W_ai^T chunk, h|e k-tiles
    wf = din("wf", (128, KT * S), BF16)        # [W_fh | W_fi]^T chunk
    wmg = din("wmg", (128, T * S), BF16)       # W_merge^T col-chunk
    wiou = din("wiou", (128, KT * 3 * S), BF16)  # W_iou^T chunk, x|mh k-tiles

    out_h = nc.dram_tensor("out_h", [128, T], F32, kind="ExternalOutput")
    out_c = nc.dram_tensor("out_c", [128, T], F32, kind="ExternalOutput")
    dbg_t = {}
    if dbg:
        for nm, shp in [("d_s96", [1, 96]), ("d_ml", [128, T]),
                        ("d_mh", [128, T]), ("d_vec4", [128, 4 * T]),
                        ("d_f", [N, S]),
                        ("d_cl", [128, T]), ("d_rst", [1, 6])]:
            dbg_t[nm] = nc.dram_tensor(nm, shp, F32, kind="ExternalOutput")

    with tile.TileContext(nc) as tc:
        with (
            tc.tile_pool(name="sb", bufs=1) as sb,
            tc.tile_pool(name="ps", bufs=1, space="PSUM") as ps,
            tc.tile_pool(name="dram", bufs=1, space="DRAM") as dram,
        ):
            # ------- warmup collective: absorbs comm-init cold cost ----
            warm_in = dram.tile([1, 64], F32, name="warm_in")
            warm_out = dram.tile([8, 64], F32, name="warm_out")
            warm_sb = sb.tile([1, 64], F32, name="warm_sb")
            nc.vector.memset(warm_sb[:], 0.0)
            nc.sync.dma_start(warm_in[:], warm_sb[:])
            nc.gpsimd.collective_compute(
                "AllGather", ALU.bypass,
                replica_groups=[list(range(NC))],
                ins=[warm_in.opt()], outs=[warm_out.opt()])

            # ---------------- small resident loads ----------------
            def load(t_dram, shape, dt=F32):
                t_sb = sb.tile(shape, dt, name=t_dram.name + "_sb")
                nc.sync.dma_start(t_sb[:], t_dram[:])
                return t_sb

            hN_sb = load(hN, [128, T, N], BF16)
            xN32_sb = load(xN32, [128, T, N], BF16)
            eN32_sb = load(eN32, [128, T, N], BF16)
            x1N_sb = load(x1N, [128, T], BF16)
            cellsh_sb = load(cells_half, [N, S])
            gf_sb = load(gf_rep, [N, S])
            bfh_sb = load(bf_half, [N, S])
            wat_sb = load(wattn_rep, [N, S])
            gm_sb = load(gm, [128, T])
            bm_sb = load(bm, [128, T])
            gi_sb = load(gi, [128, T])
            bih_sb = load(bi_h, [128, T])
            go_sb = load(go, [128, T])
            boh_sb = load(bo_h, [128, T])
            gu_sb = load(gu, [128, T])
            bu_sb = load(bu, [128, T])
            gc_sb = load(gc, [128, T])
            bc_sb = load(bc, [128, T])
            ones8_sb = load(ones8, [8, 1])
            ones32_sb = load(ones32, [N, 1])
            ones128_sb = load(ones128, [128, 1])
            onesr_sb = load(onesr, [1, 128])
            ones11_sb = load(ones11, [1, 1])

            # preload the single activation table set (exp/tanh/square)
            tl_scr = sb.tile([1, 1], F32, name="tl_scr")
            nc.vector.memset(tl_scr[:], 0.5)
            nc.scalar.activation(tl_scr[:], tl_scr[:], AF.Exp)

            # table-free rsqrt: bit-trick seed (<=3.5% err) + Newton steps
            # on the DVE; 1 step -> <=1.8e-3 rel err, plenty for the 2e-2
            # budget and ~2us cheaper than an ACT_TABLE_LOAD round-trip
            def rsqrt_dve(out, x_ap, shape, nm, iters=1):
                t = sb.tile(shape, F32, name=nm + "_t")
                nc.vector.tensor_scalar(
                    out[:].bitcast(I32), x_ap.bitcast(I32), 1, -1,
                    op0=ALU.logical_shift_right, op1=ALU.bitwise_xor)
                nc.vector.tensor_scalar_add(out[:].bitcast(I32),
                                            out[:].bitcast(I32), MAGIC + 1)
                for _ in range(iters):
                    nc.vector.tensor_tensor(t[:], out[:], out[:], op=ALU.mult)
                    nc.vector.tensor_tensor(t[:], t[:], x_ap, op=ALU.mult)
                    nc.vector.tensor_scalar(t[:], t[:], -0.5, 1.5,
                                            op0=ALU.mult, op1=ALU.add)
                    nc.vector.tensor_tensor(out[:], out[:], t[:], op=ALU.mult)

            # ---------------- weight streaming DMAs (ordered) ----------------
            wai_sb = sb.tile([128, KT * S], BF16, name="wai_sb")
            wf_sb = sb.tile([128, KT * S], BF16, name="wf_sb")
            wmg_sb = sb.tile([128, T * S], BF16, name="wmg_sb")
            wiou_sb = sb.tile([128, KT * 3 * S], BF16, name="wiou_sb")

            wdmas = []
            for k in range(2):  # wai: 2 x 1MB
                wdmas.append(nc.sync.dma_start(
                    wai_sb[:, k * 4096:(k + 1) * 4096],
                    wai[:, k * 4096:(k + 1) * 4096]))
            for k in range(2):  # wf: 2 x 1MB
                wdmas.append(nc.sync.dma_start(
                    wf_sb[:, k * 4096:(k + 1) * 4096],
                    wf[:, k * 4096:(k + 1) * 4096]))
            wdmas.append(nc.sync.dma_start(wmg_sb[:], wmg[:]))  # 1MB
            for k in range(6):  # wiou: x half then mh half, 6 x 1MB
                wdmas.append(nc.sync.dma_start(
                    wiou_sb[:, k * 4096:(k + 1) * 4096],
                    wiou[:, k * 4096:(k + 1) * 4096]))
            for i in range(2, len(wdmas)):
                add_dep_helper(wdmas[i].ins, wdmas[i - 2].ins, sync=True,
                               reason="weight DMA arrival order")

            # ---------------- attention: ai, partial logits ----------------
            ps_ai = ps.tile([N, S], F32, name="ps_ai", tag="pA")
            for kt in range(KT):
                act = hN_sb if kt < T else eN32_sb
                nc.tensor.matmul(ps_ai[:], act[:, kt % T, :],
                                 wai_sb[:, kt * S:(kt + 1) * S],
                                 start=(kt == 0), stop=(kt == KT - 1))
            ai_sb = sb.tile([N, S], F32, name="ai_sb")
            nc.scalar.activation(ai_sb[:], ps_ai[:], AF.Tanh)
            aw_sb = sb.tile([N, S], F32, name="aw_sb")
            st3 = sb.tile([N, 3], F32, name="st3")
            nc.vector.tensor_tensor(aw_sb[:], ai_sb[:], wat_sb[:], op=ALU.mult)
            nc.vector.tensor_reduce(st3[:, 0:1], aw_sb[:],
                                    mybir.AxisListType.X, ALU.add)

            # ---------------- f_lin + per-child stat partials ----------------
            ps_f = ps.tile([N, S], F32, name="ps_f", tag="pB")
            for kt in range(KT):
                act = hN_sb if kt < T else xN32_sb
                nc.tensor.matmul(ps_f[:], act[:, kt % T, :],
                                 wf_sb[:, kt * S:(kt + 1) * S],
                                 start=(kt == 0), stop=(kt == KT - 1))
            f_lin_sb = sb.tile([N, S], F32, name="f_lin_sb")
            fsq_scr = sb.tile([N, S], F32, name="fsq_scr")
            nc.vector.tensor_copy(f_lin_sb[:], ps_f[:])
            nc.vector.tensor_reduce(st3[:, 1:2], f_lin_sb[:],
                                    mybir.AxisListType.X, ALU.add)
            nc.vector.scalar_tensor_tensor(fsq_scr[:], f_lin_sb[:], 1.0,
                                           f_lin_sb[:], op0=ALU.mult,
                                           op1=ALU.mult,
                                           accum_out=st3[:, 2:3])

            # speculative per-child merge projections (col-parallel W_merge):
            # M[n, s] = sum_k h[n, k] * W_merge[c*S+s, k]
            ps_M = ps.tile([N, S], F32, name="ps_M", tag="pC")
            for kt in range(T):
                nc.tensor.matmul(ps_M[:], hN_sb[:, kt, :],
                                 wmg_sb[:, kt * S:(kt + 1) * S],
                                 start=(kt == 0), stop=(kt == T - 1))
            M_sb = sb.tile([N, S], F32, name="M_sb")
            nc.vector.tensor_copy(M_sb[:], ps_M[:])

            # iou x-half: runs as soon as those weights land (PE idle time)
            ps_iou = ps.tile([1, 3 * S], F32, name="ps_iou", tag="pIOU")
            nslices = ((0, 512), (512, 768))

            def iou_mm(kt, lhs, start, stop):
                for c0, c1 in nslices:
                    nc.tensor.matmul(ps_iou[:, c0:c1], lhs,
                                     wiou_sb[:, kt * 768 + c0:kt * 768 + c1],
                                     start=start, stop=stop)

            for kt in range(T):
                iou_mm(kt, x1N_sb[:, kt:kt + 1], kt == 0, False)

            # ---------------- AG1: logits + f stats (96 floats) ----------------
            ag1_in = dram.tile([1, 3 * N], F32, name="ag1_in")
            ag1_out = dram.tile([8, 3 * N], F32, name="ag1_out")
            nc.gpsimd.dma_start(
                ag1_in[0, :].rearrange("(k n) -> n k", n=N), st3[:])
            nc.gpsimd.collective_compute(
                "AllGather", ALU.bypass,
                replica_groups=[list(range(NC))],
                ins=[ag1_in.opt()], outs=[ag1_out.opt()])
            ag1_sb = sb.tile([8, 3 * N], F32, name="ag1_sb")
            nc.sync.dma_start(ag1_sb[:], ag1_out[:])

            if CHOP == 1:
                zz = sb.tile([128, T], F32, name="zz")
                nc.vector.memset(zz[:], 0.0)
                nc.vector.tensor_copy(zz[0:8, 0:12], ag1_sb[:, 0:12])
                nc.sync.dma_start(out_c[:], zz[:])
                nc.sync.dma_start(out_h[:], zz[:])
                raise _Chopped()

            # sum partials across cores -> [1, 96] = [logits | fsum | fss]
            ps96 = ps.tile([1, 3 * N], F32, name="ps96", tag="pA")
            nc.tensor.matmul(ps96[:], ones8_sb[:], ag1_sb[:],
                             start=True, stop=True)
            # softmax without max-subtraction or normalization: the scale
            # cancels inside the merge LayerNorm
            exps_row = sb.tile([1, N], F32, name="exps_row")
            nc.scalar.activation(exps_row[:], ps96[:, 0:N], AF.Exp)
            s96 = sb.tile([1, 3 * N], F32, name="s96")
            nc.vector.tensor_copy(s96[:], ps96[:])
            # transpose [1,32] -> [32,1] via a K=1 matmul
            ps_e32 = ps.tile([N, 1], F32, name="ps_e32", tag="pE")
            nc.tensor.matmul(ps_e32[:], exps_row[:], ones11_sb[:],
                             start=True, stop=True)
            e32 = sb.tile([N, 1], F32, name="e32")
            nc.vector.tensor_copy(e32[:], ps_e32[:])

            # ml chunk = sum_n exps[n] * M[n, :] as one K=32 matmul
            ps_ml = ps.tile([1, S], F32, name="ps_ml", tag="pC")
            nc.tensor.matmul(ps_ml[:], e32[:], M_sb[:],
                             start=True, stop=True)
            mlc = sb.tile([1, S], F32, name="mlc")
            nc.vector.tensor_copy(mlc[:], ps_ml[:])

            # ---------------- AG2: merge-linear chunks ----------------
            ag2_in = dram.tile([1, S], F32, name="ag2_in")
            ag2_out = dram.tile([8, S], F32, name="ag2_out")
            nc.gpsimd.dma_start(ag2_in[:], mlc[:])
            nc.gpsimd.collective_compute(
                "AllGather", ALU.bypass,
                replica_groups=[list(range(NC))],
                ins=[ag2_in.opt()], outs=[ag2_out.opt()])
            ml_t = sb.tile([128, T], F32, name="ml_t")
            nc.sync.dma_start(
                ml_t[:],
                ag2_out[:].rearrange("qh w -> (qh w)").rearrange(
                    "(qh ql j) -> (qh ql) j", qh=8, ql=16, j=16))

            if CHOP == 2:
                zz = sb.tile([128, T], F32, name="zz")
                nc.vector.tensor_copy(zz[:], ml_t[:])
                nc.sync.dma_start(out_c[:], zz[:])
                nc.sync.dma_start(out_h[:], zz[:])
                raise _Chopped()

            # ---- f gate (off critical path): tanh half-angle sigmoid ----
            fmean = sb.tile([N, 1], F32, name="fmean")
            fvar = sb.tile([N, 1], F32, name="fvar")
            frsth = sb.tile([N, 1], F32, name="frsth")
            ps_f3 = ps.tile([N, 2], F32, name="ps_f3", tag="pC")
            nc.tensor.matmul(ps_f3[:, 0:1], s96[:, N:2 * N], ones11_sb[:],
                             start=True, stop=True)
            nc.tensor.matmul(ps_f3[:, 1:2], s96[:, 2 * N:3 * N], ones11_sb[:],
                             start=True, stop=True)
            nc.vector.tensor_scalar_mul(fmean[:], ps_f3[:, 0:1], INV_H)
            nc.vector.tensor_scalar_mul(fvar[:], ps_f3[:, 1:2], INV_H)
            fm2 = sb.tile([N, 1], F32, name="fm2")
            nc.vector.tensor_tensor(fm2[:], fmean[:], fmean[:], op=ALU.mult)
            nc.vector.tensor_sub(fvar[:], fvar[:], fm2[:])
            nc.vector.tensor_scalar_add(fvar[:], fvar[:], EPS)
            rsqrt_dve(frsth, fvar[:], [N, 1], "frs")
            nc.vector.tensor_scalar_mul(frsth[:], frsth[:], 0.5)
            ft = sb.tile([N, S], F32, name="ft")
            nc.vector.tensor_scalar(ft[:], f_lin_sb[:], fmean[:], frsth[:],
                                    op0=ALU.subtract, op1=ALU.mult)
            nc.vector.tensor_tensor(ft[:], ft[:], gf_sb[:], op=ALU.mult)
            nc.vector.tensor_tensor(ft[:], ft[:], bfh_sb[:], op=ALU.add)
            tf_sb = sb.tile([N, S], F32, name="tf_sb")
            nc.scalar.activation(tf_sb[:], ft[:], AF.Tanh)
            # fprod = 0.5*(1+tanh)*cells = (tanh + 1) * cells_half
            fprod = sb.tile([N, S], F32, name="fprod")
            nc.vector.scalar_tensor_tensor(fprod[:], tf_sb[:], 1.0,
                                           cellsh_sb[:], op0=ALU.add,
                                           op1=ALU.mult)
            # AG3 payload, per-rank order (ql, g, j) so the gathered result
            # DMAs straight into [128, (g j)] compute layout
            big = sb.tile([1, 1024], F32, name="big")
            big4 = big[:, :].rearrange("one (ql g j) -> one ql g j",
                                       ql=16, g=4)
            ps_fc = ps.tile([1, S], F32, name="ps_fc", tag="pB")
            nc.tensor.matmul(ps_fc[:], ones32_sb[:], fprod[:],
                             start=True, stop=True)
            nc.vector.tensor_copy(
                big4[:, :, 3, :],
                ps_fc[:, :].rearrange("one (ql j) -> one ql j", ql=16))

            # ---------------- post-AG2: merge LN + mh + iou mh-half ----------
            st2m = sb.tile([128, 2], F32, name="st2m")
            sqm = sb.tile([128, T], F32, name="sqm")
            nc.vector.tensor_reduce(st2m[:, 0:1], ml_t[:],
                                    mybir.AxisListType.X, ALU.add)
            nc.vector.scalar_tensor_tensor(sqm[:], ml_t[:], 1.0, ml_t[:],
                                           op0=ALU.mult, op1=ALU.mult,
                                           accum_out=st2m[:, 1:2])
            ps_sm = ps.tile([1, 2], F32, name="ps_sm", tag="pC")
            nc.tensor.matmul(ps_sm[:], ones128_sb[:], st2m[:],
                             start=True, stop=True)
            mrm = sb.tile([1, 2], F32, name="mrm")
            nc.vector.tensor_scalar_mul(mrm[:, 0:1], ps_sm[:, 0:1], INV_H)
            mvar = sb.tile([1, 1], F32, name="mvar")
            mm2 = sb.tile([1, 1], F32, name="mm2")
            nc.vector.tensor_scalar_mul(mvar[:], ps_sm[:, 1:2], INV_H)
            nc.vector.tensor_tensor(mm2[:], mrm[:, 0:1], mrm[:, 0:1],
                                    op=ALU.mult)
            nc.vector.tensor_sub(mvar[:], mvar[:], mm2[:])
            mrst = sb.tile([1, 1], F32, name="mrst")
            rsqrt_dve(mrst, mvar[:], [1, 1], "mrs")
            nc.vector.tensor_copy(mrm[:, 1:2], mrst[:])
            ps_bm = ps.tile([128, 2], F32, name="ps_bm", tag="pE")
            nc.tensor.matmul(ps_bm[:], onesr_sb[:], mrm[:],
                             start=True, stop=True)
            mrbm = sb.tile([128, 2], F32, name="mrbm")
            nc.vector.tensor_copy(mrbm[:], ps_bm[:])
            mt1 = sb.tile([128, T], F32, name="mt1")
            nc.vector.tensor_scalar(mt1[:], ml_t[:], mrbm[:, 0:1],
                                    mrbm[:, 1:2], op0=ALU.subtract,
                                    op1=ALU.mult)
            nc.vector.tensor_tensor(mt1[:], mt1[:], gm_sb[:], op=ALU.mult)
            nc.vector.tensor_tensor(mt1[:], mt1[:], bm_sb[:], op=ALU.add)
            mh_bf = sb.tile([128, T], BF16, name="mh_bf")
            nc.scalar.activation(mh_bf[:], mt1[:], AF.Tanh)

            # iou mh-half: the only post-AG2 PE work
            for kt in range(T, KT):
                iou_mm(kt, mh_bf[:, kt - T:kt - T + 1], False, kt == KT - 1)
            nc.vector.tensor_copy(
                big4[:, :, 0:3, :],
                ps_iou[:, :].rearrange("one (g ql j) -> one ql g j",
                                       g=3, ql=16))

            # ---------------- AG3: iou + fc (1024 floats) ------------
            ag3_in = dram.tile([1, 1024], F32, name="ag3_in")
            ag3_out = dram.tile([8, 1024], F32, name="ag3_out")
            nc.gpsimd.dma_start(ag3_in[:], big[:])
            nc.gpsimd.collective_compute(
                "AllGather", ALU.bypass,
                replica_groups=[list(range(NC))],
                ins=[ag3_in.opt()], outs=[ag3_out.opt()])

            # gather straight into compute layout: [128, 64] = i|o|u|fc
            vec4 = sb.tile([128, 4 * T], F32, name="vec4")
            nc.sync.dma_start(
                vec4[:],
                ag3_out[:].rearrange("qh w -> (qh w)").rearrange(
                    "(qh ql g j) -> (qh ql) (g j)", qh=8, ql=16, g=4, j=16))

            if CHOP == 3:
                zz = sb.tile([128, T], F32, name="zz")
                nc.vector.tensor_copy(zz[:], vec4[:, 0:T])
                nc.sync.dma_start(out_c[:], zz[:])
                nc.sync.dma_start(out_h[:], zz[:])
                raise _Chopped()

            def vg(g):
                return vec4[:, g * T:(g + 1) * T]

            # global iou stats from the gathered values
            st6 = sb.tile([128, 6], F32, name="st6")
            sq6 = sb.tile([128, T], F32, name="sq6")
            for g in range(3):
                nc.vector.tensor_reduce(st6[:, 2 * g:2 * g + 1], vg(g),
                                        mybir.AxisListType.X, ALU.add)
                nc.vector.scalar_tensor_tensor(
                    sq6[:], vg(g), 1.0, vg(g), op0=ALU.mult, op1=ALU.mult,
                    accum_out=st6[:, 2 * g + 1:2 * g + 2])
            ps6 = ps.tile([1, 6], F32, name="ps6", tag="pA")
            nc.tensor.matmul(ps6[:], ones128_sb[:], st6[:],
                             start=True, stop=True)
            m3 = sb.tile([1, 3], F32, name="m3")
            v3 = sb.tile([1, 3], F32, name="v3")
            m32 = sb.tile([1, 3], F32, name="m32")
            nc.vector.tensor_scalar_mul(m3[:], ps6[:, 0:6:2], INV_H)
            nc.vector.tensor_scalar_mul(v3[:], ps6[:, 1:6:2], INV_H)
            nc.vector.tensor_tensor(m32[:], m3[:], m3[:], op=ALU.mult)
            nc.vector.tensor_sub(v3[:], v3[:], m32[:])
            r3 = sb.tile([1, 3], F32, name="r3")
            rsqrt_dve(r3, v3[:], [1, 3], "r3")
            # halve rstd for i and o (tanh half-angle sigmoid)
            mr6 = sb.tile([1, 6], F32, name="mr6")
            nc.vector.tensor_copy(mr6[:, 0:6:2], m3[:])
            nc.vector.tensor_scalar_mul(mr6[:, 1:4:2], r3[:, 0:2], 0.5)
            nc.vector.tensor_copy(mr6[:, 5:6], r3[:, 2:3])
            ps_b6 = ps.tile([128, 6], F32, name="ps_b6", tag="pB")
            nc.tensor.matmul(ps_b6[:], onesr_sb[:], mr6[:],
                             start=True, stop=True)
            mrb6 = sb.tile([128, 6], F32, name="mrb6")
            nc.vector.tensor_copy(mrb6[:], ps_b6[:])

            def gate_apply(g, g_t, b_t, nm):
                t1 = sb.tile([128, T], F32, name=nm + "_t1")
                nc.vector.tensor_scalar(t1[:], vg(g),
                                        mrb6[:, 2 * g:2 * g + 1],
                                        mrb6[:, 2 * g + 1:2 * g + 2],
                                        op0=ALU.subtract, op1=ALU.mult)
                nc.vector.tensor_tensor(t1[:], t1[:], g_t[:], op=ALU.mult)
                nc.vector.tensor_tensor(t1[:], t1[:], b_t[:], op=ALU.add)
                out = sb.tile([128, T], F32, name=nm)
                nc.scalar.activation(out[:], t1[:], AF.Tanh)
                return out

            ti_sb = gate_apply(0, gi_sb, bih_sb, "tig")
            to_sb = gate_apply(1, go_sb, boh_sb, "tog")
            u_sb = gate_apply(2, gu_sb, bu_sb, "ug")

            # cell_lin = i*u + fc, i = 0.5*(1+ti)
            cl2 = sb.tile([128, T], F32, name="cl2")
            nc.vector.scalar_tensor_tensor(cl2[:], ti_sb[:], 1.0, u_sb[:],
                                           op0=ALU.add, op1=ALU.mult)
            cell_lin = sb.tile([128, T], F32, name="cell_lin")
            nc.vector.scalar_tensor_tensor(cell_lin[:], cl2[:], 0.5, vg(3),
                                           op0=ALU.mult, op1=ALU.add)

            # cell LayerNorm
            st2c = sb.tile([128, 2], F32, name="st2c")
            sqc = sb.tile([128, T], F32, name="sqc")
            nc.vector.tensor_reduce(st2c[:, 0:1], cell_lin[:],
                                    mybir.AxisListType.X, ALU.add)
            nc.vector.scalar_tensor_tensor(sqc[:], cell_lin[:], 1.0,
                                           cell_lin[:], op0=ALU.mult,
                                           op1=ALU.mult,
                                           accum_out=st2c[:, 1:2])
            ps_sc = ps.tile([1, 2], F32, name="ps_sc", tag="pC")
            nc.tensor.matmul(ps_sc[:], ones128_sb[:], st2c[:],
                             start=True, stop=True)
            mrc = sb.tile([1, 2], F32, name="mrc")
            nc.vector.tensor_scalar_mul(mrc[:, 0:1], ps_sc[:, 0:1], INV_H)
            cvar = sb.tile([1, 1], F32, name="cvar")
            cm2 = sb.tile([1, 1], F32, name="cm2")
            nc.vector.tensor_scalar_mul(cvar[:], ps_sc[:, 1:2], INV_H)
            nc.vector.tensor_tensor(cm2[:], mrc[:, 0:1], mrc[:, 0:1],
                                    op=ALU.mult)
            nc.vector.tensor_sub(cvar[:], cvar[:], cm2[:])
            crst = sb.tile([1, 1], F32, name="crst")
            rsqrt_dve(crst, cvar[:], [1, 1], "crs")
            nc.vector.tensor_copy(mrc[:, 1:2], crst[:])
            ps_bc2 = ps.tile([128, 2], F32, name="ps_bc2", tag="pE")
            nc.tensor.matmul(ps_bc2[:], onesr_sb[:], mrc[:],
                             start=True, stop=True)
            mrbc = sb.tile([128, 2], F32, name="mrbc")
            nc.vector.tensor_copy(mrbc[:], ps_bc2[:])
            ct1 = sb.tile([128, T], F32, name="ct1")
            nc.vector.tensor_scalar(ct1[:], cell_lin[:], mrbc[:, 0:1],
                                    mrbc[:, 1:2], op0=ALU.subtract,
                                    op1=ALU.mult)
            nc.vector.tensor_tensor(ct1[:], ct1[:], gc_sb[:], op=ALU.mult)
            new_c = sb.tile([128, T], F32, name="new_c")
            nc.vector.tensor_tensor(new_c[:], ct1[:], bc_sb[:], op=ALU.add)
            nc.sync.dma_start(out_c[:], new_c[:])

            th = sb.tile([128, T], F32, name="th")
            nc.scalar.activation(th[:], new_c[:], AF.Tanh)
            # new_h = o * th, o = 0.5*(1+to)
            nh2 = sb.tile([128, T], F32, name="nh2")
            nc.vector.scalar_tensor_tensor(nh2[:], to_sb[:], 1.0, th[:],
                                           op0=ALU.add, op1=ALU.mult)
            nc.sync.dma_start(out_h[:], nh2[:])  # = 2*new_h; halved on host

            if dbg:
                def dump(nm, src, shape):
                    dd = sb.tile(shape, F32, name=nm + "_d")
                    nc.vector.tensor_copy(dd[:], src)
                    nc.sync.dma_start(dbg_t[nm][:], dd[:])
                dump("d_s96", s96[:], [1, 96])
                dump("d_ml", ml_t[:], [128, T])
                dump("d_mh", mh_bf[:], [128, T])
                dump("d_vec4", vec4[:], [128, 4 * T])
                dump("d_f", tf_sb[:], [N, S])
                dump("d_cl", cell_lin[:], [128, T])
                dump("d_rst", mr6[:], [1, 6])

    nc.compile()
    return nc


def _nat(v):
    """[2048] vector -> natural [128,16] image (sb[q,j] = v[q*16+j])."""
    return np.ascontiguousarray(np.asarray(v, np.float32).reshape(128, T))


def _kpack(w_out_in, segs):
    """Pack W[out_cols, K] (torch layout) as rhs k-tiles [128, n_tiles*cols].

    segs: list of (k0, k1) contraction segments, each of length 2048; tile
    j within a segment covers contraction rows {k0 + q*16 + j : q}.
    """
    cols = w_out_in.shape[0]
    parts = []
    for (k0, k1) in segs:
        wt = np.ascontiguousarray(w_out_in[:, k0:k1].T)  # [2048, cols]
        parts.append(wt.reshape(128, T, cols))
    arr = np.concatenate(parts, axis=1)  # [128, n_tiles, cols]
    return np.ascontiguousarray(arr.reshape(128, -1)).astype(NPBF)


def kernel(input, hiddens, cells, external,
           W_ai, W_attn, W_merge, W_iou, W_fi, W_fh,
           g_merge, b_merge, g_f, b_f, g_i, b_i, g_o, b_o, g_u, b_u,
           g_c, b_c, _dbg=False):
    key = ("nc", _dbg)
    if key not in _CACHE:
        _CACHE[key] = _build(_dbg)
    nc = _CACHE[key]

    f32 = np.float32
    input = np.asarray(input, f32)
    hiddens = np.asarray(hiddens, f32)
    cells = np.asarray(cells, f32)
    external = np.asarray(external, f32)
    W_ai = np.asarray(W_ai, f32)
    W_attn = np.asarray(W_attn, f32)
    W_merge = np.asarray(W_merge, f32)
    W_iou = np.asarray(W_iou, f32)
    W_fi = np.asarray(W_fi, f32)
    W_fh = np.asarray(W_fh, f32)

    # natural k-tile packs of the replicated activations
    hNp = np.ascontiguousarray(
        hiddens.T.reshape(128, T, N)).astype(NPBF).reshape(128, T * N)
    xN32p = np.ascontiguousarray(
        np.tile(input[:, None], (1, N)).reshape(128, T, N)
    ).astype(NPBF).reshape(128, T * N)
    eN32p = np.ascontiguousarray(
        np.tile(external[:, None], (1, N)).reshape(128, T, N)
    ).astype(NPBF).reshape(128, T * N)

    com = {
        "hN": hNp, "xN32": xN32p, "eN32": eN32p,
        "x1N": input.reshape(128, T).astype(NPBF),
        "gm": _nat(g_merge), "bm": _nat(b_merge),
        "gi": _nat(g_i), "bi_h": _nat(np.asarray(b_i, f32) * 0.5),
        "go": _nat(g_o), "bo_h": _nat(np.asarray(b_o, f32) * 0.5),
        "gu": _nat(g_u), "bu": _nat(b_u),
        "gc": _nat(g_c), "bc": _nat(b_c),
        "ones8": np.ones((8, 1), f32), "ones32": np.ones((N, 1), f32),
        "ones128": np.ones((128, 1), f32), "onesr": np.ones((1, 128), f32),
        "ones11": np.ones((1, 1), f32),
    }

    Wf_cat = np.concatenate([W_fh, W_fi], axis=1)              # [H, 4096]
    in_maps = []
    for c in range(NC):
        r = slice(c * S, (c + 1) * S)
        iou_rows = np.concatenate(
            [W_iou[g * H + c * S:g * H + (c + 1) * S, :] for g in range(3)],
            axis=0)                                            # [768, 4096]
        m = dict(com)
        m.update({
            "cells_half": np.ascontiguousarray(0.5 * cells[:, r]),
            "gf_rep": np.tile(np.asarray(g_f, f32)[r], (N, 1)),
            "bf_half": np.tile(np.asarray(b_f, f32)[r] * 0.5, (N, 1)),
            "wattn_rep": np.tile(W_attn[0, r], (N, 1)),
            "wai": _kpack(W_ai[r], [(0, H), (H, 2 * H)]),
            "wf": _kpack(Wf_cat[r], [(0, H), (H, 2 * H)]),
            "wmg": _kpack(W_merge[r], [(0, H)]),
            "wiou": _kpack(iou_rows, [(0, H), (H, 2 * H)]),
        })
        in_maps.append({k: (np.ascontiguousarray(v)
                            if v.dtype in (NPBF, NPF8)
                            else np.ascontiguousarray(v, f32))
                        for k, v in m.items()})

    res = run_bass_kernel_spmd(nc, in_maps, core_ids=list(range(NC)))
    _CACHE["last_results"] = res
    r0 = res.results[0]
    new_h = (r0["out_h"].reshape(H) * 0.5).astype(f32)
    new_c = r0["out_c"].reshape(H).astype(f32)
    if _dbg:
        _CACHE["dbg"] = {k: r0[k] for k in r0 if k.startswith("d_")}
    return new_h, new_c
